# revision 18
# baseline (speedup 1.0000x reference)
"""Diffeomorphic transform (scaling-and-squaring, TIME_STEP=7) on 8 TRN2 cores.

Single SPMD program runs all 7 squaring steps on-device:
  per step: AllGather z-shards -> full channel-minor volume `vol`;
  a DVE pass builds an interleaved volume C (12 floats per (z,y,x): the
  four (y,z)-corner 3-channel values), so ONE 24-float indirect-DMA
  descriptor fetches all 8 trilinear corners of a voxel; per 128 x M
  voxel chunk the DVE computes clipped coords, floors (round-to-nearest
  of x-0.5), lerp weights and a single block offset; a Fori loop issues
  M 128-descriptor gathers; the DVE does the trilinear combine + adds f;
  stores land in the next shard buffer.  No host round-trips.
"""
import sys
sys.path.insert(0, '/opt/trn_rl_repo')
from contextlib import ExitStack
import numpy as np
import concourse.bass as bass
from concourse import mybir
from concourse import bass2jax
from concourse.bass_utils import run_bass_kernel_spmd
import jax
import jax.numpy as jnp
from functools import partial
from jax.experimental.shard_map import shard_map
from jax.sharding import Mesh, NamedSharding, PartitionSpec

F32 = mybir.dt.float32
F16 = mybir.dt.float16
I32 = mybir.dt.int32
Alu = mybir.AluOpType

_CACHE = {}
STEPS = 7
NSEM = 16


def build(D, H, W, n_cores, M, T, loop_gathers=True, steps=STEPS, debug=False, paranoid=False):
    C = 3
    NB = D * H * W              # blocks (voxels) in full volume
    NN = NB * C                 # floats in full channel-minor volume
    shd = D // n_cores
    V = shd * H * W             # voxels per core
    V3 = V * C
    NCH = V // (128 * M)        # chunks per step
    assert NCH * 128 * M == V and M % W == 0 and V3 % 128 == 0
    rpp = M // W                # (y,z) rows per partition per chunk
    NT = NB // (128 * T)        # C-build tiles per step
    assert NT * 128 * T == NB
    CB = NB * 12                # floats in interleaved volume
    VPAD = 3 * (H * W + W) + 3 * T + 64
    sx, sy, sz = 0.5 * (W - 1), 0.5 * (H - 1), 0.5 * (D - 1)
    TCH = steps * NCH           # total chunks

    nc = bass.Bass(num_devices=n_cores, detect_race_conditions=False)
    fsh = nc.dram_tensor("fsh", [V3, 1], F32, kind="ExternalInput")
    bgx = nc.dram_tensor("bgx", [128, M], F32, kind="ExternalInput")
    bgy = nc.dram_tensor("bgy", [128, rpp * NCH], F32, kind="ExternalInput")
    bgz = nc.dram_tensor("bgz", [128, rpp * NCH], F32, kind="ExternalInput")
    outp = nc.dram_tensor("outp", [V3, 1], F16, kind="ExternalOutput")
    if debug:
        dvol = nc.dram_tensor("dvol", [NN, 1], F32, kind="ExternalOutput")
        dcv = nc.dram_tensor("dcv", [CB, 1], F32, kind="ExternalOutput")
        di0 = nc.dram_tensor("di0", [128 * M, 1], I32, kind="ExternalOutput")
        dwz = nc.dram_tensor("dwz", [128 * M, 1], F32, kind="ExternalOutput")
        dwy = nc.dram_tensor("dwy", [128 * M, 1], F32, kind="ExternalOutput")
        dwx = nc.dram_tensor("dwx", [128 * M, 1], F32, kind="ExternalOutput")
        dgb = nc.dram_tensor("dgb", [128 * M * 24, 1], F32, kind="ExternalOutput")
        dfc = nc.dram_tensor("dfc", [128 * M * 3, 1], F32, kind="ExternalOutput")

    shA = nc.dram_tensor("shA", [V3, 1], F32)
    shB = nc.dram_tensor("shB", [V3, 1], F32)
    vol = nc.dram_tensor("vol", [NN + VPAD, 1], F32, addr_space="Shared")
    cvol = nc.dram_tensor("cvol", [CB, 1], F32)

    def sh_store(s):            # store target of step s
        return outp if s == steps - 1 else (shB if s % 2 == 0 else shA)

    def sh_load(s):             # f source of step s
        return fsh if s == 0 else sh_store(s - 1)

    def ag_in(s):               # AllGather input before step s
        return shA if s == 0 else sh_store(s - 1)

    with ExitStack() as ctx:
        def sb(nm, shape, dt):
            return ctx.enter_context(nc.sbuf_tensor(nm, shape, dt))
        bx = sb("bx", [128, M], F32)
        by = sb("by", [128, rpp * NCH], F32)
        bz = sb("bz", [128, rpp * NCH], F32)
        fch = [sb(f"fch{i}", [128, M * C], F32) for i in range(2)]
        out3 = [sb(f"out3{i}", [128, M * C], F32) for i in range(2)]
        gbuf = [sb(f"gbuf{i}", [128, M * 24], F32) for i in range(2)]
        i0b = [sb(f"i0b{i}", [128, M], I32) for i in range(2)]
        wzb = [sb(f"wzb{i}", [128, M], F32) for i in range(2)]
        wyb = [sb(f"wyb{i}", [128, M], F32) for i in range(2)]
        wxb = [sb(f"wxb{i}", [128, M], F32) for i in range(2)]
        zf = sb("zf", [128, M], F32)
        yf = sb("yf", [128, M], F32)
        xf = sb("xf", [128, M], F32)
        t0 = sb("t0", [128, M], F32)
        t1 = sb("t1", [128, M], F32)
        oa = sb("oa", [128, M], F32)
        ti = sb("ti", [128, M], I32)
        xls = [sb(f"xl{i}", [128, M * C], F32) for i in range(4)]
        cin = [sb(f"cin{i}", [128, 12 * T], F32) for i in range(2)]
        cout = sb("cout", [128, 12 * T], F32)
        out3h = [sb(f"out3h{i}", [128, M * C], F16) for i in range(2)]

        lsem = ctx.enter_context(nc.semaphore("lsem"))
        asem = ctx.enter_context(nc.semaphore("asem"))
        bsem = ctx.enter_context(nc.semaphore("bsem"))
        ssem = ctx.enter_context(nc.semaphore("ssem"))
        ccsem = ctx.enter_context(nc.semaphore("ccsem"))
        cpsem = ctx.enter_context(nc.semaphore("cpsem"))
        clsem = ctx.enter_context(nc.semaphore("clsem"))
        cdsem = ctx.enter_context(nc.semaphore("cdsem"))
        cssem = ctx.enter_context(nc.semaphore("cssem"))
        gsems = [[ctx.enter_context(nc.semaphore(f"gsem{sidx}_{par}"))
                  for par in range(2)] for sidx in range(steps)]
        block = ctx.enter_context(nc.Block())

        def g_sem(t):    # sem tracking chunk t's gathers
            return gsems[t // NCH][t % 2]

        def g_done(t):   # its value once chunk t's gathers land
            ck = t % NCH
            return (ck // 2 + 1) * 16 * M

        dbg_pairs = []
        if debug:
            dbg_pairs = [
                (dvol, vol[0:NN, 0].rearrange("(p m) -> p m", p=128)),
                (dcv, cvol[0:CB, 0].rearrange("(p m) -> p m", p=128)),
                (di0, i0b[0][:, :]),
                (dwz, wzb[0][:, :]),
                (dwy, wyb[0][:, :]),
                (dwx, wxb[0][:, :]),
                (dgb, gbuf[0][:, :]),
                (dfc, fch[0][:, :]),
            ]

        # ---------------- sync engine: all loads ----------------
        @block.sync
        def _(sy):
            for s in range(steps):
                sy.wait_ge(ccsem, s + 1)           # AllGather s done
                for u in range(NT):
                    gu = s * NT + u
                    if gu >= 2:
                        sy.wait_ge(cdsem, gu - 1)  # cin buf free
                    base = 3 * (u * 128 * T)
                    for zs in range(2):
                        sy.dma_start(
                            out=cin[gu % 2][:, zs * 6 * T:(zs + 1) * 6 * T],
                            in_=bass.AP(vol, base + zs * 3 * H * W,
                                        [[3 * T, 128], [3 * W, 2],
                                         [1, 3 * T]]),
                        ).then_inc(clsem, 16)
                for ck in range(NCH):
                    t = s * NCH + ck
                    if t >= 2:
                        sy.wait_ge(bsem, t - 1)    # fch buf free
                    off = ck * 128 * M * C
                    sy.dma_start(
                        out=fch[t % 2][:, :],
                        in_=sh_load(s)[off:off + 128 * M * C, 0]
                            .rearrange("(p m) -> p m", p=128),
                    ).then_inc(lsem, 16)

        # ---------------- scalar engine: all stores ----------------
        @block.scalar
        def _(sc):
            for s in range(steps):
                for u in range(NT):
                    gu = s * NT + u
                    sc.wait_ge(cdsem, gu + 1)
                    base = u * 128 * T * 12
                    sc.dma_start(
                        out=cvol[base:base + 128 * T * 12, 0]
                            .rearrange("(p m) -> p m", p=128),
                        in_=cout[:, :],
                    ).then_inc(cssem, 16)
                for ck in range(NCH):
                    t = s * NCH + ck
                    sc.wait_ge(bsem, t + 1)
                    off = ck * 128 * M * C
                    sc.dma_start(
                        out=sh_store(s)[off:off + 128 * M * C, 0]
                            .rearrange("(p m) -> p m", p=128),
                        in_=(out3h if s == steps - 1 else out3)[t % 2][:, :],
                    ).then_inc(ssem, 16)
            sc.wait_ge(ssem, 16 * NCH * steps)
            if debug:
                dsem_n = 0
                for o, src_ap in dbg_pairs:
                    sc.dma_start(out=o[:, 0].rearrange("(p m) -> p m", p=128)
                                 if o.shape[1] == 1 else o[:, :],
                                 in_=src_ap).then_inc(cpsem, 16)
                    dsem_n += 16
                sc.wait_ge(cpsem, 16 * 4 + dsem_n)

        # ---------------- vector engine: C-build + phases A/B ----------------
        def emit_A(ve, t, s, ck):
            b = t % 2
            ve.wait_ge(lsem, 16 * (t + 1))
            if t >= 2:   # i0b buf consumed by gathers(t-2)
                ve.wait_ge(g_sem(t - 2), g_done(t - 2))
            for (c, scl, lim, fr, wr, bgt) in (
                    (0, sz, D - 1, zf, wzb[b], bz),
                    (1, sy, H - 1, yf, wyb[b], by),
                    (2, sx, W - 1, xf, wxb[b], bx)):
                # t0 = f_c * scale   (strided in0, proven tensor_scalar)
                ve.tensor_scalar(out=t0[:, :], in0=fch[b][:, c::C],
                                 scalar1=float(scl), scalar2=None,
                                 op0=Alu.mult)
                # t1 = t0 + bg      (3D view + stride-0 bcast, proven)
                if c < 2:
                    ve.tensor_tensor(
                        out=t1[:, :].rearrange("p (q x) -> p q x", q=rpp),
                        in0=t0[:, :].rearrange("p (q x) -> p q x", q=rpp),
                        in1=bgt[:, ck * rpp:(ck + 1) * rpp]
                            .unsqueeze(-1).broadcast_to([128, rpp, W]),
                        op=Alu.add)
                else:
                    ve.tensor_tensor(out=t1[:, :], in0=t0[:, :],
                                     in1=bx[:, :], op=Alu.add)
                # t0 = clip(t1, 0, lim)
                ve.tensor_scalar(out=t0[:, :], in0=t1[:, :], scalar1=0.0,
                                 scalar2=float(lim), op0=Alu.max, op1=Alu.min)
                # ti = i32(t0 - 0.5)  (round-to-nearest-even == floor)
                ve.tensor_scalar(out=ti[:, :], in0=t0[:, :], scalar1=-0.5,
                                 scalar2=None, op0=Alu.add)
                # fr = f32(ti)
                ve.tensor_scalar(out=fr[:, :], in0=ti[:, :], scalar1=1,
                                 scalar2=None, op0=Alu.mult)
                # wr = t0 - fr
                ve.scalar_tensor_tensor(out=wr[:, :], in0=fr[:, :],
                                        scalar=-1.0, in1=t0[:, :],
                                        op0=Alu.mult, op1=Alu.add)
            ve.scalar_tensor_tensor(out=oa[:, :], in0=zf[:, :],
                                    scalar=float(H), in1=yf[:, :],
                                    op0=Alu.mult, op1=Alu.add)
            ve.scalar_tensor_tensor(out=t1[:, :], in0=oa[:, :],
                                    scalar=float(W), in1=xf[:, :],
                                    op0=Alu.mult, op1=Alu.add)
            # block index fits f32 exactly (< 2^23); the x12 scale does not,
            # so cast to i32 first and scale with an integer multiply.
            ve.tensor_scalar(out=ti[:, :], in0=t1[:, :], scalar1=0.0,
                             scalar2=None, op0=Alu.add)
            ve.tensor_scalar(out=i0b[b][:, :], in0=ti[:, :], scalar1=12,
                             scalar2=None, op0=Alu.mult)
            ve.barrier()
            ve.sem_inc(asem, 1)

        def emit_B(ve, t):
            b = t % 2
            final = (t // NCH == steps - 1)
            ve.wait_ge(g_sem(t), g_done(t))
            if t >= 2:
                ve.wait_ge(ssem, 16 * (t - 1))      # out3 buf free
            g3 = gbuf[b][:, :].rearrange("p (m k) -> p m k", k=24)
            wx3 = wxb[b][:, :].unsqueeze(-1).broadcast_to([128, M, C])
            wy3 = wyb[b][:, :].unsqueeze(-1).broadcast_to([128, M, C])
            wz3 = wzb[b][:, :].unsqueeze(-1).broadcast_to([128, M, C])
            for s4 in range(4):
                a = g3[:, :, 3 * s4:3 * s4 + 3]
                bb = g3[:, :, 12 + 3 * s4:12 + 3 * s4 + 3]
                xl = xls[s4][:, :].rearrange("p (m c) -> p m c", c=C)
                ve.tensor_tensor(out=xl, in0=bb, in1=a, op=Alu.subtract)
                ve.tensor_tensor(out=xl, in0=xl, in1=wx3, op=Alu.mult)
                ve.tensor_tensor(out=xl, in0=xl, in1=a, op=Alu.add)
            for (hi, lo) in ((1, 0), (3, 2)):
                xh = xls[hi][:, :].rearrange("p (m c) -> p m c", c=C)
                xo = xls[lo][:, :].rearrange("p (m c) -> p m c", c=C)
                ve.tensor_tensor(out=xh, in0=xh, in1=xo, op=Alu.subtract)
                ve.tensor_tensor(out=xh, in0=xh, in1=wy3, op=Alu.mult)
                ve.tensor_tensor(out=xh, in0=xh, in1=xo, op=Alu.add)
            x3 = xls[3][:, :].rearrange("p (m c) -> p m c", c=C)
            x1 = xls[1][:, :].rearrange("p (m c) -> p m c", c=C)
            ve.tensor_tensor(out=x3, in0=x3, in1=x1, op=Alu.subtract)
            ve.tensor_tensor(out=x3, in0=x3, in1=wz3, op=Alu.mult)
            ve.tensor_tensor(out=x3, in0=x3, in1=x1, op=Alu.add)
            ve.tensor_tensor(out=(out3h if final else out3)[b][:, :],
                             in0=xls[3][:, :],
                             in1=fch[b][:, :], op=Alu.add)
            ve.barrier()
            ve.sem_inc(bsem, 1)

        @block.vector
        def _(rve):
            if paranoid:
                class SafeVE:
                    def __init__(self, eng, sem):
                        self.eng, self.sem, self.n = eng, sem, 0
                    def _w(self, name, *a, **k):
                        self.eng.wait_ge(self.sem, self.n)
                        ins = getattr(self.eng, name)(*a, **k)
                        ins.then_inc(self.sem, 1)
                        self.n += 1
                        return ins
                    def tensor_scalar(self, *a, **k):
                        return self._w('tensor_scalar', *a, **k)
                    def tensor_tensor(self, *a, **k):
                        return self._w('tensor_tensor', *a, **k)
                    def scalar_tensor_tensor(self, *a, **k):
                        return self._w('scalar_tensor_tensor', *a, **k)
                    def wait_ge(self, *a):
                        return self.eng.wait_ge(*a)
                    def sem_inc(self, *a):
                        return self.eng.sem_inc(*a)
                    def barrier(self):
                        return self.eng.wait_ge(self.sem, self.n)
                vser = ctx.enter_context(nc.semaphore("vser"))
                ve = SafeVE(rve, vser)
            else:
                ve = rve
            ve.wait_ge(cpsem, 16 * 4)   # init copy + bx/by/bz loaded
            for s in range(steps):
                if s > 0:               # finish prev step's last chunk first
                    emit_B(ve, s * NCH - 1)
                for u in range(NT):     # C-build interleave
                    gu = s * NT + u
                    ve.wait_ge(clsem, 32 * (gu + 1))
                    if gu >= 1:
                        ve.wait_ge(cssem, 16 * gu)  # cout stored
                    ve.tensor_scalar(
                        out=cout[:, :],
                        in0=cin[gu % 2][:, :].rearrange(
                            "p (s t c) -> p t s c", s=4, t=T, c=C),
                        scalar1=1.0, scalar2=None, op0=Alu.mult,
                    )
                    ve.barrier()
                    ve.sem_inc(cdsem, 1)
                for ck in range(NCH):
                    t = s * NCH + ck
                    emit_A(ve, t, s, ck)
                    if ck >= 1:
                        emit_B(ve, t - 1)
            emit_B(ve, TCH - 1)

        # ------- gpsimd: init copy, AllGathers, gather pair-loops -------
        assert NCH % 2 == 0

        @block.gpsimd
        def _(gp):
            gp.dma_start(out=shA[:, 0].rearrange("(p m) -> p m", p=128),
                         in_=fsh[:, 0].rearrange("(p m) -> p m", p=128)
                         ).then_inc(cpsem, 16)
            gp.dma_start(out=bx[:, :], in_=bgx[:, :]).then_inc(cpsem, 16)
            gp.dma_start(out=by[:, :], in_=bgy[:, :]).then_inc(cpsem, 16)
            gp.dma_start(out=bz[:, :], in_=bgz[:, :]).then_inc(cpsem, 16)
            gp.wait_ge(cpsem, 16 * 4)

            def chunk_gathers(b, gsem):
                for jv in range(M):
                    gp.indirect_dma_start(
                        out=gbuf[b][:, jv * 24:(jv + 1) * 24],
                        out_offset=None,
                        in_=cvol[:, :],
                        in_offset=bass.IndirectOffsetOnAxis(
                            ap=i0b[b][:, jv:jv + 1], axis=0),
                    ).then_inc(gsem, 16)

            for s in range(steps):
                if s > 0:
                    gp.wait_ge(ssem, 16 * NCH * s)
                gp.collective_compute(
                    "AllGather", Alu.bypass,
                    replica_groups=[list(range(n_cores))],
                    ins=[ag_in(s)[:, :]],
                    outs=[vol[0:NN, :]],
                ).then_inc(ccsem, 1)
                gp.wait_ge(ccsem, s + 1)
                gp.wait_ge(cssem, 16 * NT * (s + 1))  # C built
                for ck in range(NCH):
                    t = s * NCH + ck
                    gp.wait_ge(asem, t + 1)           # idx ready
                    if t >= 2:
                        gp.wait_ge(bsem, t - 1)       # gbuf free
                    chunk_gathers(t % 2, gsems[s][t % 2])
            gp.wait_ge(ssem, 16 * NCH * steps)
    return nc


def _bg_arrays(D, H, W, n_cores, M, NCH, core):
    rpp = M // W
    shd = D // n_cores
    bgx = np.tile(np.arange(W, dtype=np.float32), M // W)[None, :].repeat(128, 0)
    r = (np.arange(NCH)[None, None, :] * 128 * rpp
         + np.arange(128)[:, None, None] * rpp
         + np.arange(rpp)[None, :, None])          # [128, rpp, NCH]
    r = r.transpose(0, 2, 1).reshape(128, NCH * rpp)
    H_ = H
    bgy = (r % H_).astype(np.float32)
    bgz = (r // H_ + core * shd).astype(np.float32)
    return (np.ascontiguousarray(bgx), np.ascontiguousarray(bgy),
            np.ascontiguousarray(bgz))


_EXEC = {}


def _get_exec(key, D, H, W, n_cores, M, T):
    """Build the sharded jitted executor ONCE (mirrors
    bass2jax.run_bass_via_pjrt); later calls skip tracing/executable load."""
    if key in _EXEC:
        return _EXEC[key]
    nc = build(D, H, W, n_cores, M, T, paranoid=True)
    bass2jax.install_neuronx_cc_hook()
    partition_name = (nc.partition_id_tensor.name
                      if nc.partition_id_tensor else None)
    in_names, out_names, out_avals, zero_shapes = [], [], [], []
    for alloc in nc.m.functions[0].allocations:
        if not isinstance(alloc, mybir.MemoryLocationSet):
            continue
        name = alloc.memorylocations[0].name
        if alloc.kind == "ExternalInput":
            if name != partition_name:
                in_names.append(name)
        elif alloc.kind == "ExternalOutput":
            shape = tuple(alloc.tensor_shape)
            dtype = mybir.dt.np(alloc.dtype)
            out_names.append(name)
            out_avals.append(jax.core.ShapedArray(shape, dtype))
            zero_shapes.append((shape, dtype))
    n_params = len(in_names)
    n_outs = len(out_avals)
    all_in_names = list(in_names) + list(out_names)
    if partition_name is not None:
        all_in_names.append(partition_name)
    donate = tuple(range(n_params, n_params + n_outs))

    def _body(*args):
        operands = list(args)
        if partition_name is not None:
            operands.append(bass2jax.partition_id_tensor())
        outs = bass2jax._bass_exec_p.bind(
            *operands,
            out_avals=tuple(out_avals),
            in_names=tuple(all_in_names),
            out_names=tuple(out_names),
            lowering_input_output_aliases=(),
            sim_require_finite=True,
            sim_require_nnan=True,
            nc=nc,
        )
        return tuple(outs)

    devices = jax.devices()[:n_cores]
    mesh = Mesh(np.asarray(devices), ("core",))
    in_specs = (PartitionSpec("core"),) * (n_params + n_outs)
    out_specs = (PartitionSpec("core"),) * n_outs
    sharded = jax.jit(
        shard_map(_body, mesh=mesh, in_specs=in_specs, out_specs=out_specs,
                  check_rep=False),
        donate_argnums=donate, keep_unused=True)
    shz = NamedSharding(mesh, PartitionSpec("core"))
    zero_makers = [
        jax.jit(partial(jnp.zeros, (n_cores * sh[0], *sh[1:]), dt),
                out_shardings=shz)
        for sh, dt in zero_shapes]
    # static base-grid inputs: concat once
    shd = D // n_cores
    NCH = shd * H * W // (128 * M)
    bgs = [_bg_arrays(D, H, W, n_cores, M, NCH, k) for k in range(n_cores)]
    static_in = {
        "bgx": np.concatenate([b[0] for b in bgs], axis=0),
        "bgy": np.concatenate([b[1] for b in bgs], axis=0),
        "bgz": np.concatenate([b[2] for b in bgs], axis=0),
    }
    entry = (sharded, in_names, out_names, out_avals, zero_makers, static_in)
    _EXEC[key] = entry
    return entry


def run(flow, D, H, W, n_cores, M, T, **kw):
    C = 3
    key = (D, H, W, n_cores, M, T)
    sharded, in_names, out_names, out_avals, zero_makers, static_in = _get_exec(
        key, D, H, W, n_cores, M, T)

    f = np.asarray(flow[0], dtype=np.float32) / 128.0
    fcm = np.ascontiguousarray(f.transpose(1, 2, 3, 0)).reshape(-1)
    concat_in = [fcm.reshape(-1, 1) if nm == "fsh" else static_in[nm]
                 for nm in in_names]
    concat_zeros = [zm() for zm in zero_makers]
    out_arrs = sharded(*concat_in, *concat_zeros)
    oi = out_names.index("outp")
    per = out_avals[oi].shape
    out = np.asarray(out_arrs[oi]).reshape(-1).astype(np.float32)
    full = out.reshape(D, H, W, C)
    return np.ascontiguousarray(full.transpose(3, 0, 1, 2))[None]


def _warmup():
    try:
        z = np.zeros((1, 3, 160, 192, 160), np.float32)
        run(z, 160, 192, 160, n_cores=8, M=160, T=384)
    except Exception:
        pass


_warmup()


def kernel(flow):
    return run(flow, 160, 192, 160, n_cores=8, M=160, T=384)


# revision 20
# speedup vs baseline: 1.0201x; 1.0201x over previous
"""Diffeomorphic transform (scaling-and-squaring, TIME_STEP=7) on 8 TRN2 cores.

Single SPMD program runs all 7 squaring steps on-device:
  per step: AllGather z-shards -> full channel-minor volume `vol`;
  a DVE pass builds an interleaved volume C (12 floats per (z,y,x): the
  four (y,z)-corner 3-channel values), so ONE 24-float indirect-DMA
  descriptor fetches all 8 trilinear corners of a voxel; per 128 x M
  voxel chunk the DVE computes clipped coords, floors (round-to-nearest
  of x-0.5), lerp weights and a single block offset; a Fori loop issues
  M 128-descriptor gathers; the DVE does the trilinear combine + adds f;
  stores land in the next shard buffer.  No host round-trips.
"""
import sys
sys.path.insert(0, '/opt/trn_rl_repo')
from contextlib import ExitStack
import numpy as np
import concourse.bass as bass
from concourse import mybir
from concourse import bass2jax
from concourse.bass_utils import run_bass_kernel_spmd
import jax
import jax.numpy as jnp
from functools import partial
from jax.experimental.shard_map import shard_map
from jax.sharding import Mesh, NamedSharding, PartitionSpec

F32 = mybir.dt.float32
F16 = mybir.dt.float16
I32 = mybir.dt.int32
Alu = mybir.AluOpType

_CACHE = {}
STEPS = 7
NSEM = 16


def build(D, H, W, n_cores, M, T, loop_gathers=True, steps=STEPS, debug=False, paranoid=False):
    C = 3
    NB = D * H * W              # blocks (voxels) in full volume
    NN = NB * C                 # floats in full channel-minor volume
    shd = D // n_cores
    V = shd * H * W             # voxels per core
    V3 = V * C
    NCH = V // (128 * M)        # chunks per step
    assert NCH * 128 * M == V and M % W == 0 and V3 % 128 == 0
    rpp = M // W                # (y,z) rows per partition per chunk
    NT = NB // (128 * T)        # C-build tiles per step
    assert NT * 128 * T == NB
    CB = NB * 12                # floats in interleaved volume
    VPAD = 3 * (H * W + W) + 3 * T + 64
    sx, sy, sz = 0.5 * (W - 1), 0.5 * (H - 1), 0.5 * (D - 1)
    TCH = steps * NCH           # total chunks

    nc = bass.Bass(num_devices=n_cores, detect_race_conditions=False)
    fsh = nc.dram_tensor("fsh", [V3, 1], F32, kind="ExternalInput")
    bgx = nc.dram_tensor("bgx", [128, M], F32, kind="ExternalInput")
    bgy = nc.dram_tensor("bgy", [128, rpp * NCH], F32, kind="ExternalInput")
    bgz = nc.dram_tensor("bgz", [128, rpp * NCH], F32, kind="ExternalInput")
    outp = nc.dram_tensor("outp", [V3, 1], F16, kind="ExternalOutput")
    if debug:
        dvol = nc.dram_tensor("dvol", [NN, 1], F32, kind="ExternalOutput")
        dcv = nc.dram_tensor("dcv", [CB, 1], F32, kind="ExternalOutput")
        di0 = nc.dram_tensor("di0", [128 * M, 1], I32, kind="ExternalOutput")
        dwz = nc.dram_tensor("dwz", [128 * M, 1], F32, kind="ExternalOutput")
        dwy = nc.dram_tensor("dwy", [128 * M, 1], F32, kind="ExternalOutput")
        dwx = nc.dram_tensor("dwx", [128 * M, 1], F32, kind="ExternalOutput")
        dgb = nc.dram_tensor("dgb", [128 * M * 24, 1], F32, kind="ExternalOutput")
        dfc = nc.dram_tensor("dfc", [128 * M * 3, 1], F32, kind="ExternalOutput")

    shA = nc.dram_tensor("shA", [V3, 1], F32)
    shB = nc.dram_tensor("shB", [V3, 1], F32)
    vol = nc.dram_tensor("vol", [NN + VPAD, 1], F32, addr_space="Shared")
    cvol = nc.dram_tensor("cvol", [CB, 1], F32)

    def sh_store(s):            # store target of step s
        return outp if s == steps - 1 else (shB if s % 2 == 0 else shA)

    def sh_load(s):             # f source of step s
        return fsh if s == 0 else sh_store(s - 1)

    def ag_in(s):               # AllGather input before step s
        return shA if s == 0 else sh_store(s - 1)

    with ExitStack() as ctx:
        def sb(nm, shape, dt):
            return ctx.enter_context(nc.sbuf_tensor(nm, shape, dt))
        bx = sb("bx", [128, M], F32)
        by = sb("by", [128, rpp * NCH], F32)
        bz = sb("bz", [128, rpp * NCH], F32)
        fch = [sb(f"fch{i}", [128, M * C], F32) for i in range(2)]
        out3 = [sb(f"out3{i}", [128, M * C], F32) for i in range(2)]
        gbuf = [sb(f"gbuf{i}", [128, M * 24], F32) for i in range(2)]
        i0b = [sb(f"i0b{i}", [128, M], I32) for i in range(2)]
        wzb = [sb(f"wzb{i}", [128, M], F32) for i in range(2)]
        wyb = [sb(f"wyb{i}", [128, M], F32) for i in range(2)]
        wxb = [sb(f"wxb{i}", [128, M], F32) for i in range(2)]
        zf = sb("zf", [128, M], F32)
        yf = sb("yf", [128, M], F32)
        xf = sb("xf", [128, M], F32)
        t0 = sb("t0", [128, M], F32)
        t1 = sb("t1", [128, M], F32)
        oa = sb("oa", [128, M], F32)
        ti = sb("ti", [128, M], I32)
        xls = [sb(f"xl{i}", [128, M * C], F32) for i in range(4)]
        cin = [sb(f"cin{i}", [128, 12 * T], F32) for i in range(2)]
        cout = sb("cout", [128, 12 * T], F32)
        out3h = [sb(f"out3h{i}", [128, M * C], F16) for i in range(2)]

        lsem = ctx.enter_context(nc.semaphore("lsem"))
        asem = ctx.enter_context(nc.semaphore("asem"))
        bsem = ctx.enter_context(nc.semaphore("bsem"))
        ssem = ctx.enter_context(nc.semaphore("ssem"))
        ccsem = ctx.enter_context(nc.semaphore("ccsem"))
        cpsem = ctx.enter_context(nc.semaphore("cpsem"))
        clsem = ctx.enter_context(nc.semaphore("clsem"))
        cdsem = ctx.enter_context(nc.semaphore("cdsem"))
        cssem = ctx.enter_context(nc.semaphore("cssem"))
        gsems = [[ctx.enter_context(nc.semaphore(f"gsem{sidx}_{par}"))
                  for par in range(2)] for sidx in range(steps)]
        block = ctx.enter_context(nc.Block())

        def g_sem(t):    # sem tracking chunk t's gathers
            return gsems[t // NCH][t % 2]

        def g_done(t):   # its value once chunk t's gathers land
            ck = t % NCH
            return (ck // 2 + 1) * 16 * M

        dbg_pairs = []
        if debug:
            dbg_pairs = [
                (dvol, vol[0:NN, 0].rearrange("(p m) -> p m", p=128)),
                (dcv, cvol[0:CB, 0].rearrange("(p m) -> p m", p=128)),
                (di0, i0b[0][:, :]),
                (dwz, wzb[0][:, :]),
                (dwy, wyb[0][:, :]),
                (dwx, wxb[0][:, :]),
                (dgb, gbuf[0][:, :]),
                (dfc, fch[0][:, :]),
            ]

        # ---------------- sync engine: all loads ----------------
        @block.sync
        def _(sy):
            for s in range(steps):
                sy.wait_ge(ccsem, s + 1)           # AllGather s done
                for u in range(NT):
                    gu = s * NT + u
                    if gu >= 2:
                        sy.wait_ge(cdsem, gu - 1)  # cin buf free
                    base = 3 * (u * 128 * T)
                    for zs in range(2):
                        sy.dma_start(
                            out=cin[gu % 2][:, zs * 6 * T:(zs + 1) * 6 * T],
                            in_=bass.AP(vol, base + zs * 3 * H * W,
                                        [[3 * T, 128], [3 * W, 2],
                                         [1, 3 * T]]),
                        ).then_inc(clsem, 16)
                for ck in range(NCH):
                    t = s * NCH + ck
                    if t >= 2:
                        sy.wait_ge(bsem, t - 1)    # fch buf free
                    off = ck * 128 * M * C
                    sy.dma_start(
                        out=fch[t % 2][:, :],
                        in_=sh_load(s)[off:off + 128 * M * C, 0]
                            .rearrange("(p m) -> p m", p=128),
                    ).then_inc(lsem, 16)

        # ---------------- scalar engine: all stores ----------------
        @block.scalar
        def _(sc):
            for s in range(steps):
                for u in range(NT):
                    gu = s * NT + u
                    sc.wait_ge(cdsem, gu + 1)
                    base = u * 128 * T * 12
                    sc.dma_start(
                        out=cvol[base:base + 128 * T * 12, 0]
                            .rearrange("(p m) -> p m", p=128),
                        in_=cout[:, :],
                    ).then_inc(cssem, 16)
                for ck in range(NCH):
                    t = s * NCH + ck
                    sc.wait_ge(bsem, t + 1)
                    off = ck * 128 * M * C
                    sc.dma_start(
                        out=sh_store(s)[off:off + 128 * M * C, 0]
                            .rearrange("(p m) -> p m", p=128),
                        in_=(out3h if s == steps - 1 else out3)[t % 2][:, :],
                    ).then_inc(ssem, 16)
            sc.wait_ge(ssem, 16 * NCH * steps)
            if debug:
                dsem_n = 0
                for o, src_ap in dbg_pairs:
                    sc.dma_start(out=o[:, 0].rearrange("(p m) -> p m", p=128)
                                 if o.shape[1] == 1 else o[:, :],
                                 in_=src_ap).then_inc(cpsem, 16)
                    dsem_n += 16
                sc.wait_ge(cpsem, 16 * 4 + dsem_n)

        # ---------------- vector engine: C-build + phases A/B ----------------
        def emit_A(ve, t, s, ck):
            b = t % 2
            ve.wait_ge(lsem, 16 * (t + 1))
            if t >= 2:   # i0b buf consumed by gathers(t-2)
                ve.wait_ge(g_sem(t - 2), g_done(t - 2))
            for (c, scl, lim, fr, wr, bgt) in (
                    (0, sz, D - 1, zf, wzb[b], bz),
                    (1, sy, H - 1, yf, wyb[b], by),
                    (2, sx, W - 1, xf, wxb[b], bx)):
                # t0 = f_c * scale   (strided in0, proven tensor_scalar)
                ve.tensor_scalar(out=t0[:, :], in0=fch[b][:, c::C],
                                 scalar1=float(scl), scalar2=None,
                                 op0=Alu.mult)
                # t1 = t0 + bg      (3D view + stride-0 bcast, proven)
                if c < 2:
                    ve.tensor_tensor(
                        out=t1[:, :].rearrange("p (q x) -> p q x", q=rpp),
                        in0=t0[:, :].rearrange("p (q x) -> p q x", q=rpp),
                        in1=bgt[:, ck * rpp:(ck + 1) * rpp]
                            .unsqueeze(-1).broadcast_to([128, rpp, W]),
                        op=Alu.add)
                else:
                    ve.tensor_tensor(out=t1[:, :], in0=t0[:, :],
                                     in1=bx[:, :], op=Alu.add)
                # t0 = clip(t1, 0, lim)
                ve.tensor_scalar(out=t0[:, :], in0=t1[:, :], scalar1=0.0,
                                 scalar2=float(lim), op0=Alu.max, op1=Alu.min)
                # ti = i32(t0 - 0.5)  (round-to-nearest-even == floor)
                ve.tensor_scalar(out=ti[:, :], in0=t0[:, :], scalar1=-0.5,
                                 scalar2=None, op0=Alu.add)
                # fr = f32(ti)
                ve.tensor_scalar(out=fr[:, :], in0=ti[:, :], scalar1=1,
                                 scalar2=None, op0=Alu.mult)
                # wr = t0 - fr
                ve.scalar_tensor_tensor(out=wr[:, :], in0=fr[:, :],
                                        scalar=-1.0, in1=t0[:, :],
                                        op0=Alu.mult, op1=Alu.add)
            ve.scalar_tensor_tensor(out=oa[:, :], in0=zf[:, :],
                                    scalar=float(H), in1=yf[:, :],
                                    op0=Alu.mult, op1=Alu.add)
            ve.scalar_tensor_tensor(out=t1[:, :], in0=oa[:, :],
                                    scalar=float(W), in1=xf[:, :],
                                    op0=Alu.mult, op1=Alu.add)
            # block index fits f32 exactly (< 2^23); the x12 scale does not,
            # so cast to i32 first and scale with an integer multiply.
            ve.tensor_scalar(out=ti[:, :], in0=t1[:, :], scalar1=0.0,
                             scalar2=None, op0=Alu.add)
            ve.tensor_scalar(out=i0b[b][:, :], in0=ti[:, :], scalar1=12,
                             scalar2=None, op0=Alu.mult)
            ve.barrier()
            ve.sem_inc(asem, 1)

        def emit_B(ve, t):
            b = t % 2
            final = (t // NCH == steps - 1)
            ve.wait_ge(g_sem(t), g_done(t))
            if t >= 2:
                ve.wait_ge(ssem, 16 * (t - 1))      # out3 buf free
            g3 = gbuf[b][:, :].rearrange("p (m k) -> p m k", k=24)
            wx3 = wxb[b][:, :].unsqueeze(-1).broadcast_to([128, M, C])
            wy3 = wyb[b][:, :].unsqueeze(-1).broadcast_to([128, M, C])
            wz3 = wzb[b][:, :].unsqueeze(-1).broadcast_to([128, M, C])
            for s4 in range(4):
                a = g3[:, :, 3 * s4:3 * s4 + 3]
                bb = g3[:, :, 12 + 3 * s4:12 + 3 * s4 + 3]
                xl = xls[s4][:, :].rearrange("p (m c) -> p m c", c=C)
                ve.tensor_tensor(out=xl, in0=bb, in1=a, op=Alu.subtract)
                ve.tensor_tensor(out=xl, in0=xl, in1=wx3, op=Alu.mult)
                ve.tensor_tensor(out=xl, in0=xl, in1=a, op=Alu.add)
            for (hi, lo) in ((1, 0), (3, 2)):
                xh = xls[hi][:, :].rearrange("p (m c) -> p m c", c=C)
                xo = xls[lo][:, :].rearrange("p (m c) -> p m c", c=C)
                ve.tensor_tensor(out=xh, in0=xh, in1=xo, op=Alu.subtract)
                ve.tensor_tensor(out=xh, in0=xh, in1=wy3, op=Alu.mult)
                ve.tensor_tensor(out=xh, in0=xh, in1=xo, op=Alu.add)
            x3 = xls[3][:, :].rearrange("p (m c) -> p m c", c=C)
            x1 = xls[1][:, :].rearrange("p (m c) -> p m c", c=C)
            ve.tensor_tensor(out=x3, in0=x3, in1=x1, op=Alu.subtract)
            ve.tensor_tensor(out=x3, in0=x3, in1=wz3, op=Alu.mult)
            ve.tensor_tensor(out=x3, in0=x3, in1=x1, op=Alu.add)
            ve.tensor_tensor(out=(out3h if final else out3)[b][:, :],
                             in0=xls[3][:, :],
                             in1=fch[b][:, :], op=Alu.add)
            ve.barrier()
            ve.sem_inc(bsem, 1)

        @block.vector
        def _(rve):
            if paranoid:
                class SafeVE:
                    def __init__(self, eng, sem):
                        self.eng, self.sem, self.n = eng, sem, 0
                    def _w(self, name, *a, **k):
                        self.eng.wait_ge(self.sem, self.n)
                        ins = getattr(self.eng, name)(*a, **k)
                        ins.then_inc(self.sem, 1)
                        self.n += 1
                        return ins
                    def tensor_scalar(self, *a, **k):
                        return self._w('tensor_scalar', *a, **k)
                    def tensor_tensor(self, *a, **k):
                        return self._w('tensor_tensor', *a, **k)
                    def scalar_tensor_tensor(self, *a, **k):
                        return self._w('scalar_tensor_tensor', *a, **k)
                    def wait_ge(self, *a):
                        return self.eng.wait_ge(*a)
                    def sem_inc(self, *a):
                        return self.eng.sem_inc(*a)
                    def barrier(self):
                        return self.eng.wait_ge(self.sem, self.n)
                vser = ctx.enter_context(nc.semaphore("vser"))
                ve = SafeVE(rve, vser)
            else:
                ve = rve
            ve.wait_ge(cpsem, 16 * 4)   # init copy + bx/by/bz loaded
            for s in range(steps):
                if s > 0:               # finish prev step's last chunk first
                    emit_B(ve, s * NCH - 1)
                for u in range(NT):     # C-build interleave
                    gu = s * NT + u
                    ve.wait_ge(clsem, 32 * (gu + 1))
                    if gu >= 1:
                        ve.wait_ge(cssem, 16 * gu)  # cout stored
                    ve.tensor_scalar(
                        out=cout[:, :],
                        in0=cin[gu % 2][:, :].rearrange(
                            "p (s t c) -> p t s c", s=4, t=T, c=C),
                        scalar1=1.0, scalar2=None, op0=Alu.mult,
                    )
                    ve.barrier()
                    ve.sem_inc(cdsem, 1)
                for ck in range(NCH):
                    t = s * NCH + ck
                    emit_A(ve, t, s, ck)
                    if ck >= 1:
                        emit_B(ve, t - 1)
            emit_B(ve, TCH - 1)

        # ------- gpsimd: init copy, AllGathers, gather pair-loops -------
        assert NCH % 2 == 0

        @block.gpsimd
        def _(gp):
            gp.dma_start(out=shA[:, 0].rearrange("(p m) -> p m", p=128),
                         in_=fsh[:, 0].rearrange("(p m) -> p m", p=128)
                         ).then_inc(cpsem, 16)
            gp.dma_start(out=bx[:, :], in_=bgx[:, :]).then_inc(cpsem, 16)
            gp.dma_start(out=by[:, :], in_=bgy[:, :]).then_inc(cpsem, 16)
            gp.dma_start(out=bz[:, :], in_=bgz[:, :]).then_inc(cpsem, 16)
            gp.wait_ge(cpsem, 16 * 4)

            def chunk_gathers(b, gsem):
                for jv in range(M):
                    gp.indirect_dma_start(
                        out=gbuf[b][:, jv * 24:(jv + 1) * 24],
                        out_offset=None,
                        in_=cvol[:, :],
                        in_offset=bass.IndirectOffsetOnAxis(
                            ap=i0b[b][:, jv:jv + 1], axis=0),
                    ).then_inc(gsem, 16)

            for s in range(steps):
                if s > 0:
                    gp.wait_ge(ssem, 16 * NCH * s)
                gp.collective_compute(
                    "AllGather", Alu.bypass,
                    replica_groups=[list(range(n_cores))],
                    ins=[ag_in(s)[:, :]],
                    outs=[vol[0:NN, :]],
                ).then_inc(ccsem, 1)
                gp.wait_ge(ccsem, s + 1)
                gp.wait_ge(cssem, 16 * NT * (s + 1))  # C built
                for ck in range(NCH):
                    t = s * NCH + ck
                    gp.wait_ge(asem, t + 1)           # idx ready
                    if t >= 2:
                        gp.wait_ge(bsem, t - 1)       # gbuf free
                    chunk_gathers(t % 2, gsems[s][t % 2])
            gp.wait_ge(ssem, 16 * NCH * steps)
    return nc


def _bg_arrays(D, H, W, n_cores, M, NCH, core):
    rpp = M // W
    shd = D // n_cores
    bgx = np.tile(np.arange(W, dtype=np.float32), M // W)[None, :].repeat(128, 0)
    r = (np.arange(NCH)[None, None, :] * 128 * rpp
         + np.arange(128)[:, None, None] * rpp
         + np.arange(rpp)[None, :, None])          # [128, rpp, NCH]
    r = r.transpose(0, 2, 1).reshape(128, NCH * rpp)
    H_ = H
    bgy = (r % H_).astype(np.float32)
    bgz = (r // H_ + core * shd).astype(np.float32)
    return (np.ascontiguousarray(bgx), np.ascontiguousarray(bgy),
            np.ascontiguousarray(bgz))


_EXEC = {}


def _get_exec(key, D, H, W, n_cores, M, T):
    """Build the sharded jitted executor ONCE (mirrors
    bass2jax.run_bass_via_pjrt); later calls skip tracing/executable load."""
    if key in _EXEC:
        return _EXEC[key]
    nc = build(D, H, W, n_cores, M, T, paranoid=True)
    bass2jax.install_neuronx_cc_hook()
    partition_name = (nc.partition_id_tensor.name
                      if nc.partition_id_tensor else None)
    in_names, out_names, out_avals, zero_shapes = [], [], [], []
    for alloc in nc.m.functions[0].allocations:
        if not isinstance(alloc, mybir.MemoryLocationSet):
            continue
        name = alloc.memorylocations[0].name
        if alloc.kind == "ExternalInput":
            if name != partition_name:
                in_names.append(name)
        elif alloc.kind == "ExternalOutput":
            shape = tuple(alloc.tensor_shape)
            dtype = mybir.dt.np(alloc.dtype)
            out_names.append(name)
            out_avals.append(jax.core.ShapedArray(shape, dtype))
            zero_shapes.append((shape, dtype))
    n_params = len(in_names)
    n_outs = len(out_avals)
    all_in_names = list(in_names) + list(out_names)
    if partition_name is not None:
        all_in_names.append(partition_name)
    donate = tuple(range(n_params, n_params + n_outs))

    def _body(*args):
        operands = list(args)
        if partition_name is not None:
            operands.append(bass2jax.partition_id_tensor())
        outs = bass2jax._bass_exec_p.bind(
            *operands,
            out_avals=tuple(out_avals),
            in_names=tuple(all_in_names),
            out_names=tuple(out_names),
            lowering_input_output_aliases=(),
            sim_require_finite=True,
            sim_require_nnan=True,
            nc=nc,
        )
        return tuple(outs)

    devices = jax.devices()[:n_cores]
    mesh = Mesh(np.asarray(devices), ("core",))
    in_specs = (PartitionSpec("core"),) * (n_params + n_outs)
    out_specs = (PartitionSpec("core"),) * n_outs
    sharded = jax.jit(
        shard_map(_body, mesh=mesh, in_specs=in_specs, out_specs=out_specs,
                  check_rep=False),
        donate_argnums=donate, keep_unused=True)
    shz = NamedSharding(mesh, PartitionSpec("core"))
    zero_makers = [
        jax.jit(partial(jnp.zeros, (n_cores * sh[0], *sh[1:]), dt),
                out_shardings=shz)
        for sh, dt in zero_shapes]
    # static base-grid inputs: concat once
    shd = D // n_cores
    NCH = shd * H * W // (128 * M)
    bgs = [_bg_arrays(D, H, W, n_cores, M, NCH, k) for k in range(n_cores)]
    static_in = {
        "bgx": np.concatenate([b[0] for b in bgs], axis=0),
        "bgy": np.concatenate([b[1] for b in bgs], axis=0),
        "bgz": np.concatenate([b[2] for b in bgs], axis=0),
    }
    entry = (sharded, in_names, out_names, out_avals, zero_makers, static_in)
    _EXEC[key] = entry
    return entry


def run(flow, D, H, W, n_cores, M, T, **kw):
    C = 3
    key = (D, H, W, n_cores, M, T)
    sharded, in_names, out_names, out_avals, zero_makers, static_in = _get_exec(
        key, D, H, W, n_cores, M, T)

    f = np.asarray(flow[0], dtype=np.float32) / 128.0
    fcm = np.ascontiguousarray(f.transpose(1, 2, 3, 0)).reshape(-1)
    concat_in = [fcm.reshape(-1, 1) if nm == "fsh" else static_in[nm]
                 for nm in in_names]
    concat_zeros = [zm() for zm in zero_makers]
    out_arrs = sharded(*concat_in, *concat_zeros)
    oi = out_names.index("outp")
    per = out_avals[oi].shape
    out = np.asarray(out_arrs[oi]).reshape(-1).astype(np.float32)
    full = out.reshape(D, H, W, C)
    return np.ascontiguousarray(full.transpose(3, 0, 1, 2))[None]


def _warmup():
    try:
        z = np.zeros((1, 3, 160, 192, 160), np.float32)
        run(z, 160, 192, 160, n_cores=8, M=160, T=384)
    except Exception:
        pass


_warmup()


def kernel(flow):
    return run(flow, 160, 192, 160, n_cores=8, M=160, T=384)


# revision 21
# speedup vs baseline: 1.0313x; 1.0110x over previous
"""Diffeomorphic transform (scaling-and-squaring, TIME_STEP=7) on 8 TRN2 cores.

Single SPMD program runs all 7 squaring steps on-device:
  per step: AllGather z-shards -> full channel-minor volume `vol`;
  a DVE pass builds an interleaved volume C (12 floats per (z,y,x): the
  four (y,z)-corner 3-channel values), so ONE 24-float indirect-DMA
  descriptor fetches all 8 trilinear corners of a voxel; per 128 x M
  voxel chunk the DVE computes clipped coords, floors (round-to-nearest
  of x-0.5), lerp weights and a single block offset; a Fori loop issues
  M 128-descriptor gathers; the DVE does the trilinear combine + adds f;
  stores land in the next shard buffer.  No host round-trips.
"""
import sys
sys.path.insert(0, '/opt/trn_rl_repo')
from contextlib import ExitStack
import numpy as np
import concourse.bass as bass
from concourse import mybir
from concourse import bass2jax
from concourse.bass_utils import run_bass_kernel_spmd
import jax
import jax.numpy as jnp
from functools import partial
from jax.experimental.shard_map import shard_map
from jax.sharding import Mesh, NamedSharding, PartitionSpec

F32 = mybir.dt.float32
F16 = mybir.dt.float16
I32 = mybir.dt.int32
Alu = mybir.AluOpType

_CACHE = {}
STEPS = 7
NSEM = 16


def build(D, H, W, n_cores, M, T, loop_gathers=True, steps=STEPS, debug=False, paranoid=False):
    C = 3
    NB = D * H * W              # blocks (voxels) in full volume
    NN = NB * C                 # floats in full channel-minor volume
    shd = D // n_cores
    V = shd * H * W             # voxels per core
    V3 = V * C
    NCH = V // (128 * M)        # chunks per step
    assert NCH * 128 * M == V and M % W == 0 and V3 % 128 == 0
    rpp = M // W                # (y,z) rows per partition per chunk
    NT = NB // (128 * T)        # C-build tiles per step
    assert NT * 128 * T == NB
    CB = NB * 12                # floats in interleaved volume
    VPAD = 3 * (H * W + W) + 3 * T + 64
    sx, sy, sz = 0.5 * (W - 1), 0.5 * (H - 1), 0.5 * (D - 1)
    TCH = steps * NCH           # total chunks

    nc = bass.Bass(num_devices=n_cores, detect_race_conditions=False)
    fsh = nc.dram_tensor("fsh", [V3, 1], F32, kind="ExternalInput")
    bgx = nc.dram_tensor("bgx", [128, M], F32, kind="ExternalInput")
    bgy = nc.dram_tensor("bgy", [128, rpp * NCH], F32, kind="ExternalInput")
    bgz = nc.dram_tensor("bgz", [128, rpp * NCH], F32, kind="ExternalInput")
    outp = nc.dram_tensor("outp", [V3, 1], F16, kind="ExternalOutput")
    if debug:
        dvol = nc.dram_tensor("dvol", [NN, 1], F32, kind="ExternalOutput")
        dcv = nc.dram_tensor("dcv", [CB, 1], F32, kind="ExternalOutput")
        di0 = nc.dram_tensor("di0", [128 * M, 1], I32, kind="ExternalOutput")
        dwz = nc.dram_tensor("dwz", [128 * M, 1], F32, kind="ExternalOutput")
        dwy = nc.dram_tensor("dwy", [128 * M, 1], F32, kind="ExternalOutput")
        dwx = nc.dram_tensor("dwx", [128 * M, 1], F32, kind="ExternalOutput")
        dgb = nc.dram_tensor("dgb", [128 * M * 24, 1], F32, kind="ExternalOutput")
        dfc = nc.dram_tensor("dfc", [128 * M * 3, 1], F32, kind="ExternalOutput")

    shA = nc.dram_tensor("shA", [V3, 1], F32)
    shB = nc.dram_tensor("shB", [V3, 1], F32)
    vol = nc.dram_tensor("vol", [NN + VPAD, 1], F32, addr_space="Shared")
    cvol = nc.dram_tensor("cvol", [CB, 1], F32)

    def sh_store(s):            # store target of step s
        return outp if s == steps - 1 else (shB if s % 2 == 0 else shA)

    def sh_load(s):             # f source of step s
        return fsh if s == 0 else sh_store(s - 1)

    def ag_in(s):               # AllGather input before step s
        return shA if s == 0 else sh_store(s - 1)

    with ExitStack() as ctx:
        def sb(nm, shape, dt):
            return ctx.enter_context(nc.sbuf_tensor(nm, shape, dt))
        bx = sb("bx", [128, M], F32)
        by = sb("by", [128, rpp * NCH], F32)
        bz = sb("bz", [128, rpp * NCH], F32)
        fch = [sb(f"fch{i}", [128, M * C], F32) for i in range(2)]
        out3 = [sb(f"out3{i}", [128, M * C], F32) for i in range(2)]
        gbuf = [sb(f"gbuf{i}", [128, M * 24], F32) for i in range(2)]
        i0b = [sb(f"i0b{i}", [128, M], I32) for i in range(2)]
        wzb = [sb(f"wzb{i}", [128, M], F32) for i in range(2)]
        wyb = [sb(f"wyb{i}", [128, M], F32) for i in range(2)]
        wxb = [sb(f"wxb{i}", [128, M], F32) for i in range(2)]
        zf = sb("zf", [128, M], F32)
        yf = sb("yf", [128, M], F32)
        xf = sb("xf", [128, M], F32)
        t0 = sb("t0", [128, M], F32)
        t1 = sb("t1", [128, M], F32)
        oa = sb("oa", [128, M], F32)
        ti = sb("ti", [128, M], I32)
        xls = [sb(f"xl{i}", [128, M * C], F32) for i in range(4)]
        cin = [sb(f"cin{i}", [128, 12 * T], F32) for i in range(2)]
        cout = sb("cout", [128, 12 * T], F32)
        out3h = [sb(f"out3h{i}", [128, M * C], F16) for i in range(2)]

        lsem = ctx.enter_context(nc.semaphore("lsem"))
        asem = ctx.enter_context(nc.semaphore("asem"))
        bsem = ctx.enter_context(nc.semaphore("bsem"))
        ssem = ctx.enter_context(nc.semaphore("ssem"))
        ccsem = ctx.enter_context(nc.semaphore("ccsem"))
        cpsem = ctx.enter_context(nc.semaphore("cpsem"))
        clsem = ctx.enter_context(nc.semaphore("clsem"))
        cdsem = ctx.enter_context(nc.semaphore("cdsem"))
        cssem = ctx.enter_context(nc.semaphore("cssem"))
        gsems = [[ctx.enter_context(nc.semaphore(f"gsem{sidx}_{par}"))
                  for par in range(2)] for sidx in range(steps)]
        block = ctx.enter_context(nc.Block())

        def g_sem(t):    # sem tracking chunk t's gathers
            return gsems[t // NCH][t % 2]

        def g_done(t):   # its value once chunk t's gathers land
            ck = t % NCH
            return (ck // 2 + 1) * 16 * M

        dbg_pairs = []
        if debug:
            dbg_pairs = [
                (dvol, vol[0:NN, 0].rearrange("(p m) -> p m", p=128)),
                (dcv, cvol[0:CB, 0].rearrange("(p m) -> p m", p=128)),
                (di0, i0b[0][:, :]),
                (dwz, wzb[0][:, :]),
                (dwy, wyb[0][:, :]),
                (dwx, wxb[0][:, :]),
                (dgb, gbuf[0][:, :]),
                (dfc, fch[0][:, :]),
            ]

        # ---------------- sync engine: all loads ----------------
        @block.sync
        def _(sy):
            for s in range(steps):
                sy.wait_ge(ccsem, s + 1)           # AllGather s done
                for u in range(NT):
                    gu = s * NT + u
                    if gu >= 2:
                        sy.wait_ge(cdsem, gu - 1)  # cin buf free
                    base = 3 * (u * 128 * T)
                    for zs in range(2):
                        sy.dma_start(
                            out=cin[gu % 2][:, zs * 6 * T:(zs + 1) * 6 * T],
                            in_=bass.AP(vol, base + zs * 3 * H * W,
                                        [[3 * T, 128], [3 * W, 2],
                                         [1, 3 * T]]),
                        ).then_inc(clsem, 16)
                for ck in range(NCH):
                    t = s * NCH + ck
                    if t >= 2:
                        sy.wait_ge(bsem, t - 1)    # fch buf free
                    off = ck * 128 * M * C
                    sy.dma_start(
                        out=fch[t % 2][:, :],
                        in_=sh_load(s)[off:off + 128 * M * C, 0]
                            .rearrange("(p m) -> p m", p=128),
                    ).then_inc(lsem, 16)

        # ---------------- scalar engine: all stores ----------------
        @block.scalar
        def _(sc):
            for s in range(steps):
                for u in range(NT):
                    gu = s * NT + u
                    sc.wait_ge(cdsem, gu + 1)
                    base = u * 128 * T * 12
                    sc.dma_start(
                        out=cvol[base:base + 128 * T * 12, 0]
                            .rearrange("(p m) -> p m", p=128),
                        in_=cout[:, :],
                    ).then_inc(cssem, 16)
                for ck in range(NCH):
                    t = s * NCH + ck
                    sc.wait_ge(bsem, t + 1)
                    off = ck * 128 * M * C
                    sc.dma_start(
                        out=sh_store(s)[off:off + 128 * M * C, 0]
                            .rearrange("(p m) -> p m", p=128),
                        in_=(out3h if s == steps - 1 else out3)[t % 2][:, :],
                    ).then_inc(ssem, 16)
            sc.wait_ge(ssem, 16 * NCH * steps)
            if debug:
                dsem_n = 0
                for o, src_ap in dbg_pairs:
                    sc.dma_start(out=o[:, 0].rearrange("(p m) -> p m", p=128)
                                 if o.shape[1] == 1 else o[:, :],
                                 in_=src_ap).then_inc(cpsem, 16)
                    dsem_n += 16
                sc.wait_ge(cpsem, 16 * 4 + dsem_n)

        # ---------------- vector engine: C-build + phases A/B ----------------
        def emit_A(ve, t, s, ck):
            b = t % 2
            ve.wait_ge(lsem, 16 * (t + 1))
            if t >= 2:   # i0b buf consumed by gathers(t-2)
                ve.wait_ge(g_sem(t - 2), g_done(t - 2))
            for (c, scl, lim, fr, wr, bgt) in (
                    (0, sz, D - 1, zf, wzb[b], bz),
                    (1, sy, H - 1, yf, wyb[b], by),
                    (2, sx, W - 1, xf, wxb[b], bx)):
                # t0 = f_c * scale   (strided in0, proven tensor_scalar)
                ve.tensor_scalar(out=t0[:, :], in0=fch[b][:, c::C],
                                 scalar1=float(scl), scalar2=None,
                                 op0=Alu.mult)
                # t1 = t0 + bg      (3D view + stride-0 bcast, proven)
                if c < 2:
                    ve.tensor_tensor(
                        out=t1[:, :].rearrange("p (q x) -> p q x", q=rpp),
                        in0=t0[:, :].rearrange("p (q x) -> p q x", q=rpp),
                        in1=bgt[:, ck * rpp:(ck + 1) * rpp]
                            .unsqueeze(-1).broadcast_to([128, rpp, W]),
                        op=Alu.add)
                else:
                    ve.tensor_tensor(out=t1[:, :], in0=t0[:, :],
                                     in1=bx[:, :], op=Alu.add)
                # t0 = clip(t1, 0, lim)
                ve.tensor_scalar(out=t0[:, :], in0=t1[:, :], scalar1=0.0,
                                 scalar2=float(lim), op0=Alu.max, op1=Alu.min)
                # ti = i32(t0 - 0.5)  (round-to-nearest-even == floor)
                ve.tensor_scalar(out=ti[:, :], in0=t0[:, :], scalar1=-0.5,
                                 scalar2=None, op0=Alu.add)
                # fr = f32(ti)
                ve.tensor_scalar(out=fr[:, :], in0=ti[:, :], scalar1=1,
                                 scalar2=None, op0=Alu.mult)
                # wr = t0 - fr
                ve.scalar_tensor_tensor(out=wr[:, :], in0=fr[:, :],
                                        scalar=-1.0, in1=t0[:, :],
                                        op0=Alu.mult, op1=Alu.add)
            ve.scalar_tensor_tensor(out=oa[:, :], in0=zf[:, :],
                                    scalar=float(H), in1=yf[:, :],
                                    op0=Alu.mult, op1=Alu.add)
            ve.scalar_tensor_tensor(out=t1[:, :], in0=oa[:, :],
                                    scalar=float(W), in1=xf[:, :],
                                    op0=Alu.mult, op1=Alu.add)
            # block index fits f32 exactly (< 2^23); the x12 scale does not,
            # so cast to i32 first and scale with an integer multiply.
            ve.tensor_scalar(out=ti[:, :], in0=t1[:, :], scalar1=0.0,
                             scalar2=None, op0=Alu.add)
            ve.tensor_scalar(out=i0b[b][:, :], in0=ti[:, :], scalar1=12,
                             scalar2=None, op0=Alu.mult)
            ve.barrier()
            ve.sem_inc(asem, 1)

        def emit_B(ve, t):
            b = t % 2
            final = (t // NCH == steps - 1)
            ve.wait_ge(g_sem(t), g_done(t))
            if t >= 2:
                ve.wait_ge(ssem, 16 * (t - 1))      # out3 buf free
            g3 = gbuf[b][:, :].rearrange("p (m k) -> p m k", k=24)
            wx3 = wxb[b][:, :].unsqueeze(-1).broadcast_to([128, M, C])
            wy3 = wyb[b][:, :].unsqueeze(-1).broadcast_to([128, M, C])
            wz3 = wzb[b][:, :].unsqueeze(-1).broadcast_to([128, M, C])
            for s4 in range(4):
                a = g3[:, :, 3 * s4:3 * s4 + 3]
                bb = g3[:, :, 12 + 3 * s4:12 + 3 * s4 + 3]
                xl = xls[s4][:, :].rearrange("p (m c) -> p m c", c=C)
                ve.tensor_tensor(out=xl, in0=bb, in1=a, op=Alu.subtract)
                ve.tensor_tensor(out=xl, in0=xl, in1=wx3, op=Alu.mult)
                ve.tensor_tensor(out=xl, in0=xl, in1=a, op=Alu.add)
            for (hi, lo) in ((1, 0), (3, 2)):
                xh = xls[hi][:, :].rearrange("p (m c) -> p m c", c=C)
                xo = xls[lo][:, :].rearrange("p (m c) -> p m c", c=C)
                ve.tensor_tensor(out=xh, in0=xh, in1=xo, op=Alu.subtract)
                ve.tensor_tensor(out=xh, in0=xh, in1=wy3, op=Alu.mult)
                ve.tensor_tensor(out=xh, in0=xh, in1=xo, op=Alu.add)
            x3 = xls[3][:, :].rearrange("p (m c) -> p m c", c=C)
            x1 = xls[1][:, :].rearrange("p (m c) -> p m c", c=C)
            ve.tensor_tensor(out=x3, in0=x3, in1=x1, op=Alu.subtract)
            ve.tensor_tensor(out=x3, in0=x3, in1=wz3, op=Alu.mult)
            ve.tensor_tensor(out=x3, in0=x3, in1=x1, op=Alu.add)
            ve.tensor_tensor(out=(out3h if final else out3)[b][:, :],
                             in0=xls[3][:, :],
                             in1=fch[b][:, :], op=Alu.add)
            ve.barrier()
            ve.sem_inc(bsem, 1)

        @block.vector
        def _(rve):
            if paranoid:
                class SafeVE:
                    def __init__(self, eng, sem):
                        self.eng, self.sem, self.n = eng, sem, 0
                    def _w(self, name, *a, **k):
                        self.eng.wait_ge(self.sem, self.n)
                        ins = getattr(self.eng, name)(*a, **k)
                        ins.then_inc(self.sem, 1)
                        self.n += 1
                        return ins
                    def tensor_scalar(self, *a, **k):
                        return self._w('tensor_scalar', *a, **k)
                    def tensor_tensor(self, *a, **k):
                        return self._w('tensor_tensor', *a, **k)
                    def scalar_tensor_tensor(self, *a, **k):
                        return self._w('scalar_tensor_tensor', *a, **k)
                    def wait_ge(self, *a):
                        return self.eng.wait_ge(*a)
                    def sem_inc(self, *a):
                        return self.eng.sem_inc(*a)
                    def barrier(self):
                        return self.eng.wait_ge(self.sem, self.n)
                vser = ctx.enter_context(nc.semaphore("vser"))
                ve = SafeVE(rve, vser)
            else:
                ve = rve
            ve.wait_ge(cpsem, 16 * 4)   # init copy + bx/by/bz loaded
            for s in range(steps):
                if s > 0:               # finish prev step's last chunk first
                    emit_B(ve, s * NCH - 1)
                for u in range(NT):     # C-build interleave
                    gu = s * NT + u
                    ve.wait_ge(clsem, 32 * (gu + 1))
                    if gu >= 1:
                        ve.wait_ge(cssem, 16 * gu)  # cout stored
                    ve.tensor_scalar(
                        out=cout[:, :],
                        in0=cin[gu % 2][:, :].rearrange(
                            "p (s t c) -> p t s c", s=4, t=T, c=C),
                        scalar1=1.0, scalar2=None, op0=Alu.mult,
                    )
                    ve.barrier()
                    ve.sem_inc(cdsem, 1)
                for ck in range(NCH):
                    t = s * NCH + ck
                    emit_A(ve, t, s, ck)
                    if ck >= 1:
                        emit_B(ve, t - 1)
            emit_B(ve, TCH - 1)

        # ------- gpsimd: init copy, AllGathers, gather pair-loops -------
        assert NCH % 2 == 0

        @block.gpsimd
        def _(gp):
            gp.dma_start(out=shA[:, 0].rearrange("(p m) -> p m", p=128),
                         in_=fsh[:, 0].rearrange("(p m) -> p m", p=128)
                         ).then_inc(cpsem, 16)
            gp.dma_start(out=bx[:, :], in_=bgx[:, :]).then_inc(cpsem, 16)
            gp.dma_start(out=by[:, :], in_=bgy[:, :]).then_inc(cpsem, 16)
            gp.dma_start(out=bz[:, :], in_=bgz[:, :]).then_inc(cpsem, 16)
            gp.wait_ge(cpsem, 16 * 4)

            def chunk_gathers(b, gsem):
                for jv in range(M):
                    gp.indirect_dma_start(
                        out=gbuf[b][:, jv * 24:(jv + 1) * 24],
                        out_offset=None,
                        in_=cvol[:, :],
                        in_offset=bass.IndirectOffsetOnAxis(
                            ap=i0b[b][:, jv:jv + 1], axis=0),
                    ).then_inc(gsem, 16)

            for s in range(steps):
                if s > 0:
                    gp.wait_ge(ssem, 16 * NCH * s)
                gp.collective_compute(
                    "AllGather", Alu.bypass,
                    replica_groups=[list(range(n_cores))],
                    ins=[ag_in(s)[:, :]],
                    outs=[vol[0:NN, :]],
                ).then_inc(ccsem, 1)
                gp.wait_ge(ccsem, s + 1)
                gp.wait_ge(cssem, 16 * NT * (s + 1))  # C built
                for ck in range(NCH):
                    t = s * NCH + ck
                    gp.wait_ge(asem, t + 1)           # idx ready
                    if t >= 2:
                        gp.wait_ge(bsem, t - 1)       # gbuf free
                    chunk_gathers(t % 2, gsems[s][t % 2])
            gp.wait_ge(ssem, 16 * NCH * steps)
    return nc


def _bg_arrays(D, H, W, n_cores, M, NCH, core):
    rpp = M // W
    shd = D // n_cores
    bgx = np.tile(np.arange(W, dtype=np.float32), M // W)[None, :].repeat(128, 0)
    r = (np.arange(NCH)[None, None, :] * 128 * rpp
         + np.arange(128)[:, None, None] * rpp
         + np.arange(rpp)[None, :, None])          # [128, rpp, NCH]
    r = r.transpose(0, 2, 1).reshape(128, NCH * rpp)
    H_ = H
    bgy = (r % H_).astype(np.float32)
    bgz = (r // H_ + core * shd).astype(np.float32)
    return (np.ascontiguousarray(bgx), np.ascontiguousarray(bgy),
            np.ascontiguousarray(bgz))


_EXEC = {}


def _get_exec(key, D, H, W, n_cores, M, T):
    """Build the sharded jitted executor ONCE (mirrors
    bass2jax.run_bass_via_pjrt); later calls skip tracing/executable load."""
    if key in _EXEC:
        return _EXEC[key]
    nc = build(D, H, W, n_cores, M, T, paranoid=True)
    bass2jax.install_neuronx_cc_hook()
    partition_name = (nc.partition_id_tensor.name
                      if nc.partition_id_tensor else None)
    in_names, out_names, out_avals, zero_shapes = [], [], [], []
    for alloc in nc.m.functions[0].allocations:
        if not isinstance(alloc, mybir.MemoryLocationSet):
            continue
        name = alloc.memorylocations[0].name
        if alloc.kind == "ExternalInput":
            if name != partition_name:
                in_names.append(name)
        elif alloc.kind == "ExternalOutput":
            shape = tuple(alloc.tensor_shape)
            dtype = mybir.dt.np(alloc.dtype)
            out_names.append(name)
            out_avals.append(jax.core.ShapedArray(shape, dtype))
            zero_shapes.append((shape, dtype))
    n_params = len(in_names)
    n_outs = len(out_avals)
    all_in_names = list(in_names) + list(out_names)
    if partition_name is not None:
        all_in_names.append(partition_name)
    donate = tuple(range(n_params, n_params + n_outs))

    def _body(*args):
        operands = list(args)
        if partition_name is not None:
            operands.append(bass2jax.partition_id_tensor())
        outs = bass2jax._bass_exec_p.bind(
            *operands,
            out_avals=tuple(out_avals),
            in_names=tuple(all_in_names),
            out_names=tuple(out_names),
            lowering_input_output_aliases=(),
            sim_require_finite=True,
            sim_require_nnan=True,
            nc=nc,
        )
        return tuple(outs)

    devices = jax.devices()[:n_cores]
    mesh = Mesh(np.asarray(devices), ("core",))
    in_specs = (PartitionSpec("core"),) * (n_params + n_outs)
    out_specs = (PartitionSpec("core"),) * n_outs
    sharded = jax.jit(
        shard_map(_body, mesh=mesh, in_specs=in_specs, out_specs=out_specs,
                  check_rep=False),
        donate_argnums=donate, keep_unused=True)
    shz = NamedSharding(mesh, PartitionSpec("core"))
    zero_makers = [
        jax.jit(partial(jnp.zeros, (n_cores * sh[0], *sh[1:]), dt),
                out_shardings=shz)
        for sh, dt in zero_shapes]
    # static base-grid inputs: concat once
    shd = D // n_cores
    NCH = shd * H * W // (128 * M)
    bgs = [_bg_arrays(D, H, W, n_cores, M, NCH, k) for k in range(n_cores)]
    static_in = {
        "bgx": np.concatenate([b[0] for b in bgs], axis=0),
        "bgy": np.concatenate([b[1] for b in bgs], axis=0),
        "bgz": np.concatenate([b[2] for b in bgs], axis=0),
    }
    entry = (sharded, in_names, out_names, out_avals, zero_makers, static_in)
    _EXEC[key] = entry
    return entry


def run(flow, D, H, W, n_cores, M, T, **kw):
    C = 3
    key = (D, H, W, n_cores, M, T)
    sharded, in_names, out_names, out_avals, zero_makers, static_in = _get_exec(
        key, D, H, W, n_cores, M, T)

    # one fused pass: strided-view transpose * 2^-7 -> contiguous result
    f4 = np.asarray(flow[0], dtype=np.float32)
    fcm = (f4.transpose(1, 2, 3, 0) * np.float32(0.0078125)).reshape(-1)
    concat_in = [fcm.reshape(-1, 1) if nm == "fsh" else static_in[nm]
                 for nm in in_names]
    concat_zeros = [zm() for zm in zero_makers]
    out_arrs = sharded(*concat_in, *concat_zeros)
    oi = out_names.index("outp")
    per = out_avals[oi].shape
    out = np.asarray(out_arrs[oi]).reshape(-1).astype(np.float32)
    full = out.reshape(D, H, W, C)
    return np.ascontiguousarray(full.transpose(3, 0, 1, 2))[None]


def _warmup():
    try:
        z = np.zeros((1, 3, 160, 192, 160), np.float32)
        run(z, 160, 192, 160, n_cores=8, M=160, T=384)
    except Exception:
        pass


_warmup()


def kernel(flow):
    return run(flow, 160, 192, 160, n_cores=8, M=160, T=384)
